# revision 10
# baseline (speedup 1.0000x reference)
"""Trainium2 Bass kernel for int8 GEMM + bias (IntLinear).

Computes y = x @ weight + bias with x:[8192,4096] int8, weight:[4096,4096] int8,
bias:[4096] int8 -> y:[8192,4096] int32.

Strategy
--------
- int8 values are exactly representable in bf16 (8-bit significand), and every
  product/partial sum of this GEMM stays far below 2^24, so a bf16 x bf16
  matmul with fp32 PSUM accumulation reproduces the int32 result bit-exactly.
- Data-parallel shard: rows of x split across 8 NeuronCores (1024 rows each);
  weight replicated.  Each core runs a tiled bf16 GEMM (XT stationary, W
  moving) at the PE roofline (215.8ns per 128x128x512 matmul).
- Host side: inputs are packed into per-tile contiguous blocks (partition-
  major), so every DMA descriptor is a 4KB contiguous run instead of 1KB --
  the DMA engines saturate ~150GB/s at 1KB/descriptor, which both throttled
  the kernel head and left no margin in steady state.
- bias is added on host in int32 (exact; bias is int8).

Perf notes (trace-driven):
- The 2048 matmuls are the hardware floor; recoverable time is the head
  (DMA prefetch before the first MM), the HAM cold-clock ramp, and the tail.
- DMA triggers (DIRECT2D) cost ~0.6us on the issuing engine: kxm triggers on
  gpsimd, kxn on sync, output copy+store on scalar, so none serializes.
- k-tiles ramp [2,2,4,...]: the first matmul gates on 256KB per side.
- Fine-grained dummy matmuls (N=128) on a zeroed tile bridge the DMA head so
  the PE's HAM clock gate is already released when real matmuls start.
- kxm tags use bufs=1 so m1's x-tiles naturally prefetch during the last m0
  block (when the n-snake reuse frees all kxn bandwidth).
- Output is copied PSUM->SBUF and DMA'd per 128-row subtile, so the kernel
  tail only gates on one 256KB store.
"""

import numpy as np
import ml_dtypes

import concourse.bass  # noqa: F401  (registers engines)
import concourse.mybir as mybir
import concourse.tile as tile
from concourse import bacc
from concourse.bass_utils import run_bass_kernel_spmd

M, K, N = 8192, 4096, 4096
N_CORES = 8
M_LOC = M // N_CORES
P = 128

# k-tile subtile counts (x128 rows each): small first tiles to un-gate the
# first matmul quickly, then coarse tiles to keep the DMA trigger count low.
K_SUBTILES = [2, 2, 4, 4, 4, 4, 4, 4, 4]
NKT = len(K_SUBTILES)
K_STARTS = [0]
for _ks in K_SUBTILES:
    K_STARTS.append(K_STARTS[-1] + _ks)
assert K_STARTS[-1] * P == K

M_TILES = M_LOC // 512          # 2 outer m tiles of 512 rows
N_TILES = N // 512              # 8 n tiles of 512 cols
M_SUB = 4                       # 128-row subtiles per m tile

WARMUP_MMS = 24                 # N=128 dummies, ~107ns each cold

_compiled = None


def _build():
    global _compiled
    if _compiled is not None:
        return _compiled

    nc = bacc.Bacc("TRN2", target_bir_lowering=False, debug=False,
                   num_devices=N_CORES)
    # Pre-tiled flat inputs: per (mo|no, kt) a contiguous [128p, ks, 512]
    # block (partition-major), so DMA descriptors are ks*1KB contiguous.
    xtp = nc.dram_tensor("xtp", [K * 512 * M_TILES], mybir.dt.bfloat16,
                         kind="ExternalInput").ap()
    wp = nc.dram_tensor("wp", [K * 512 * N_TILES], mybir.dt.bfloat16,
                        kind="ExternalInput").ap()
    y = nc.dram_tensor("y", [M_LOC, N], mybir.dt.int32,
                       kind="ExternalOutput").ap()

    def src_block(flat, outer, kt):
        ks = K_SUBTILES[kt]
        off = outer * (K * 512) + K_STARTS[kt] * P * 512
        return flat[off:off + P * ks * 512].rearrange("(p a) -> p a", p=P)

    with tile.TileContext(nc) as tc:
        tc.swap_default_side()
        with (
            tc.tile_pool(name="warm_pool", bufs=1) as warm_pool,
            tc.tile_pool(name="kxm_pool", bufs=1) as kxm_pool,
            tc.tile_pool(name="kxn_pool", bufs=2) as kxn_pool,
            tc.tile_pool(name="stage_pool", bufs=8) as stage_pool,
            tc.tile_pool(name="psum_pool", bufs=2, space="PSUM") as psum_pool,
        ):
            # --- PE warmup (HAM clock-gate release) ---
            warm = warm_pool.tile([P, P], mybir.dt.bfloat16)
            nc.gpsimd.memset(warm, 0)
            wps = psum_pool.tile([P, 512], mybir.dt.float32, tag="ps0",
                                 space="PSUM", name="warm_ps")
            for _ in range(WARMUP_MMS):
                nc.tensor.matmul(wps[:, :P], warm, warm, start=True, stop=True)

            def load_kxm(mo, kt):
                ks = K_SUBTILES[kt]
                t = kxm_pool.tile([P, ks * 512], mybir.dt.bfloat16,
                                  tag=f"kxm{kt}", name=f"kxm{kt}")
                # scalar is the second HWDGE engine (fast descriptor-gen);
                # it is otherwise idle until the first copybacks (~35us in).
                nc.scalar.dma_start(t, src_block(xtp, mo, kt))
                return t

            def load_kxn(no, kt):
                ks = K_SUBTILES[kt]
                t = kxn_pool.tile([P, ks * 512], mybir.dt.bfloat16,
                                  tag=f"kxn{kt}", name=f"kxn{kt}")
                nc.sync.dma_start(t, src_block(wp, no, kt))
                return t

            kxm_tiles = {}
            kxn_tiles = {}
            psum_tiles = {}

            for mo in range(M_TILES):
                # Snake over n so the m-turn reuses the last n's kxn tiles.
                n_order = range(N_TILES) if mo % 2 == 0 else \
                    range(N_TILES - 1, -1, -1)
                for ni, no in enumerate(n_order):
                    first_n = ni == 0
                    for kt in range(NKT):
                        if mo == 0 and first_n and kt == 1:
                            # Gate the rest of the prefetch on the first
                            # k-tile pair's arrival (1-element copies create
                            # the dependency), so the tiles gating the first
                            # real matmul get exclusive DMA bandwidth.
                            ks = K_SUBTILES[1]
                            tm = kxm_pool.tile([P, ks * 512],
                                               mybir.dt.bfloat16,
                                               tag="kxm1", name="kxm1")
                            nc.vector.tensor_copy(out=tm[:, 0:1],
                                                  in_=kxm_tiles[0][:, 0:1])
                            nc.scalar.dma_start(tm, src_block(xtp, 0, 1))
                            kxm_tiles[1] = tm
                            tn = kxn_pool.tile([P, ks * 512],
                                               mybir.dt.bfloat16,
                                               tag="kxn1", name="kxn1")
                            nc.vector.tensor_copy(out=tn[:, 0:1],
                                                  in_=kxn_tiles[0][:, 0:1])
                            nc.sync.dma_start(tn, src_block(wp, 0, 1))
                            kxn_tiles[1] = tn
                            continue
                        if first_n:
                            kxm_tiles[kt] = load_kxm(mo, kt)
                        if mo == 0 or not first_n:
                            kxn_tiles[kt] = load_kxn(no, kt)

                    for ms in range(M_SUB):
                        psum_tiles[ms] = psum_pool.tile(
                            [P, 512], mybir.dt.float32, tag=f"ps{ms}",
                            space="PSUM", name=f"ps{ms}")

                    for kt in range(NKT):
                        kxm_t = kxm_tiles[kt]
                        kxn_t = kxn_tiles[kt]
                        for ms in range(M_SUB):
                            for ksub in range(K_SUBTILES[kt]):
                                nc.tensor.matmul(
                                    psum_tiles[ms],
                                    kxm_t[:, ksub * 512 + ms * P:
                                          ksub * 512 + (ms + 1) * P],
                                    kxn_t[:, ksub * 512:(ksub + 1) * 512],
                                    start=(kt == 0 and ksub == 0),
                                    stop=(kt == NKT - 1
                                          and ksub == K_SUBTILES[kt] - 1),
                                )

                    # Drain: per-subtile copyback on scalar, store trigger on
                    # sync, so the last block's copies are not serialized
                    # behind DMA descriptor-gen on the same engine.
                    for ms in range(M_SUB):
                        st = stage_pool.tile([P, 512], mybir.dt.int32,
                                             tag="stage", name="stage")
                        nc.scalar.copy(out=st, in_=psum_tiles[ms])
                        r0 = mo * 512 + ms * P
                        c0 = no * 512
                        nc.sync.dma_start(y[r0:r0 + P, c0:c0 + 512], st)

    nc.compile()
    _compiled = nc
    return nc


def _pack_tiles(arr_kxf, f_tiles):
    """[K, F] -> flat blocks: for each f-tile (512 cols) and k-tile, a
    contiguous [128, ks, 512] partition-major block."""
    out = np.empty(arr_kxf.size, dtype=arr_kxf.dtype)
    pos = 0
    for f in range(f_tiles):
        cols = arr_kxf[:, f * 512:(f + 1) * 512]
        for kt in range(NKT):
            ks = K_SUBTILES[kt]
            blk = cols[K_STARTS[kt] * P:(K_STARTS[kt] + ks) * P, :]
            blk = blk.reshape(ks, P, 512).transpose(1, 0, 2)  # [128, ks, 512]
            n = blk.size
            out[pos:pos + n] = blk.reshape(-1)
            pos += n
    return out


def _run(x, weight, trace=False, **spmd_kwargs):
    """Run the device GEMM. Returns (y_int32 [M,N], BassKernelResults)."""
    nc = _build()
    xt_bf16 = np.ascontiguousarray(x.T).astype(ml_dtypes.bfloat16)  # [K, M]
    w_bf16 = np.asarray(weight).astype(ml_dtypes.bfloat16)          # [K, N]
    wp = _pack_tiles(w_bf16, N_TILES)
    in_maps = [
        {
            "xtp": _pack_tiles(
                xt_bf16[:, i * M_LOC:(i + 1) * M_LOC], M_TILES),
            "wp": wp,
        }
        for i in range(N_CORES)
    ]
    res = run_bass_kernel_spmd(nc, in_maps, list(range(N_CORES)),
                               trace=trace, **spmd_kwargs)
    y = np.concatenate([res.results[i]["y"] for i in range(N_CORES)], axis=0)
    return y, res


def kernel(x, weight, bias):
    y, _ = _run(np.asarray(x), np.asarray(weight))
    return y + np.asarray(bias).astype(np.int32)
